# revision 1
# baseline (speedup 1.0000x reference)
"""ChebConv (order-4) GNN layer on 8 Trainium2 NeuronCores.

Reference computation (fp32):
    T0 = x, T1 = G x, Tk = 2 G T{k-1} - T{k-2}
    out = sum_k Tk @ W[k]          # [N, F] with N=10000, F=32

Strategy:
  * Rewrite in the power basis: y0 = x, yk = G y{k-1},
      out = sum_k yk @ Wp[k]  with
      Wp = [W0 - W2, W1 - 3 W3, 2 W2, 4 W3]   (exact modulo fp reassociation)
    so each hop is a bare matmul against G (no 2*/- epilogue).
  * Row-shard G over 8 cores (1280 padded rows each). The per-core lhsT
    tiles must hold G^T, so the host passes each core a contiguous
    transposed slice (pad N 10000 -> 10240).
  * fp32 matmuls on the TRN2 PE run in LOW_HIGH mode: 2 passes, each
    streaming the fp32 rhs at half rate (4x bf16 cost). Instead we do a
    software hi/lo split: G = G_hi + G_lo and v = v_hi + v_lo (bf16
    pairs) and compute G_hi v_hi + G_lo v_hi + G_hi v_lo with fp32 PSUM
    accumulation -- 3 full-rate bf16 passes, same DRAM bytes as fp32,
    ~7e-6 relative error (vs 3e-3 for plain bf16). Per fc sweep, G_hi
    and G_lo rows are interleaved in one [NP, 2*l] array so each
    128-row j-chunk is a single contiguous DMA.
  * Each hop runs as 3 sweeps, one per <=512-column chunk of yk^T.
    Per sweep and 128-row j-chunk: matmuls (lhsT=v_{hi,lo}[j-chunk]
    [128,32] bf16, rhs=G^T_{hi,lo} tile [128,<=512] bf16) accumulate
    the sweep's [32,<=512] chunk of yk^T over all 80 j-chunks (one open
    PSUM accumulation group per bank). The last (smallest) sweep's G
    block stays pinned in SBUF across hops (10.5 MB saved twice).
  * The Wp contraction happens on-chip from yk^T in full fp32:
    matmul(lhsT=Wp_k [32,32], rhs=ykT chunk), DVE-add into the
    transposed output accumulator; the k=0 term uses the host xT slice.
  * After each sweep (except in the last hop), its rows are
    PE-transposed ([32,128] -> [128,32] blocks) into natural m-chunk
    layout, split into bf16 hi/lo, and all-gathered in a partial
    collective (DRAM bounce) that overlaps the remaining sweeps. The
    reload into the next hop's per-part v tiles rides SWDGE (gpsimd) so
    the gather-gated DMA cannot convoy the G stream on the shared HWDGE
    completion lanes; j-chunks are consumed in gather-firing order so
    each hop starts on columns whose gather finished first.
  * Output is returned transposed ([32, 1280] per core); the host
    concatenates, transposes and drops padding.
"""

import sys

if "/opt/trn_rl_repo" not in sys.path:
    sys.path.insert(0, "/opt/trn_rl_repo")

import numpy as np

N = 10000
F = 32
ORDER = 4
NCORES = 8
P = 128
NP = 10240  # padded node count: divisible by NCORES * P
RPC = NP // NCORES  # rows per core (1280)
JC = NP // P  # global 128-row chunks (80)
MC = RPC // P  # local 128-row chunks per core (10)

_CACHE = {}


def _build(np_total, ncores):
    from concourse import bacc, masks, mybir, tile

    rpc = np_total // ncores
    jc = np_total // P
    mc = rpc // P
    f32 = mybir.dt.float32
    bf16 = mybir.dt.bfloat16
    fchunks = [(s, min(512, rpc - s)) for s in range(0, rpc, 512)]
    nfc = len(fchunks)

    nc = bacc.Bacc(
        "TRN2", target_bir_lowering=False, debug=False, num_devices=ncores
    )
    # one G^T block per fc sweep, rows = [hi cols | lo cols] interleaved
    ghls = [
        nc.dram_tensor(f"ghl{i}", [np_total, 2 * l], bf16, kind="ExternalInput").ap()
        for i, (s, l) in enumerate(fchunks)
    ]
    # per-part m-chunk geometry: part i covers m-chunks [m0, m0+nm)
    parts = [(s // P, l // P) for s, l in fchunks]
    # x in per-part v layout: concat over parts of [hi block | lo block],
    # block col (c*nm + ml)*F + f = padded x row (c*mc + m0 + ml)*P + p
    vcols = [2 * ncores * nm * F for (m0, nm) in parts]
    xthl = nc.dram_tensor("xthl", [P, sum(vcols)], bf16, kind="ExternalInput").ap()
    xt = nc.dram_tensor("xt", [F, rpc], f32, kind="ExternalInput").ap()
    wp = nc.dram_tensor("wp", [F, ORDER * F], f32, kind="ExternalInput").ap()
    out_t = nc.dram_tensor("outT", [F, rpc], f32, kind="ExternalOutput").ap()

    # pin the last (smallest) fc sweep's G block in SBUF across hops
    pin_i = nfc - 1
    pin_l = fchunks[pin_i][1]

    def part_of(m):
        for i, (m0, nm) in enumerate(parts):
            if m0 <= m < m0 + nm:
                return i
        raise AssertionError

    with tile.TileContext(nc) as tc:
        with (
            tc.tile_pool(name="const", bufs=1) as constp,
            tc.tile_pool(name="gtp", bufs=10) as gtp,
            tc.tile_pool(name="vp", bufs=2) as vp,
            tc.tile_pool(name="sb", bufs=2) as sb,
            tc.tile_pool(name="ps_hop", bufs=1, space="PSUM") as ps_hop,
            tc.tile_pool(name="ps_tp", bufs=2, space="PSUM") as ps_tp,
            tc.tile_pool(name="ps_w", bufs=2, space="PSUM") as ps_w,
            tc.tile_pool(name="dram", bufs=2, space="DRAM") as dram,
        ):
            ident = constp.tile([P, P], f32)
            masks.make_identity(nc, ident[:])
            w_sb = constp.tile([F, ORDER * F], f32)
            nc.scalar.dma_start(w_sb[:], wp)
            xt_sb = constp.tile([F, rpc], f32)
            nc.scalar.dma_start(xt_sb[:], xt)
            out_sb = constp.tile([F, rpc], f32)
            pin = constp.tile([P, jc * 2 * pin_l], bf16)

            # v holds y_{k-1} as bf16 hi/lo pairs, one tile per fc part so
            # next-hop matmuls only depend on the partial gather that
            # produced their columns
            v_parts = []
            off = 0
            for i, w_ in enumerate(vcols):
                vt = vp.tile([P, w_], bf16, tag=f"v{i}", name=f"v{i}")
                nc.scalar.dma_start(vt[:], xthl[:, off : off + w_])
                off += w_
                v_parts.append(vt)

            def v_hi(vps, j):
                c, m = j // mc, j % mc
                i = part_of(m)
                m0, nm = parts[i]
                col = (c * 2 * nm + (m - m0)) * F
                return vps[i][:, col : col + F]

            def v_lo(vps, j):
                c, m = j // mc, j % mc
                i = part_of(m)
                m0, nm = parts[i]
                col = (c * 2 * nm + nm + (m - m0)) * F
                return vps[i][:, col : col + F]

            # k = 0 contribution: out^T = Wp_0^T @ x^T (pure fp32)
            for s, l in fchunks:
                pw = ps_w.tile([F, l], f32, tag="pw")
                nc.tensor.matmul(
                    pw[:], lhsT=w_sb[:, 0:F], rhs=xt_sb[:, s : s + l],
                    start=True, stop=True,
                )
                nc.vector.tensor_copy(out_sb[:, s : s + l], pw[:])

            # j-chunks are consumed in sweep (= gather-firing) order so
            # each hop starts on columns whose gather finished first; the
            # pinned sweep stays last: its gather is smallest and its
            # consumers come after a ~46us runway in the next hop
            sweep_order = list(range(nfc))
            part_rank = {i: r for r, i in enumerate(sweep_order)}
            jorder = sorted(range(jc), key=lambda j: (part_rank[part_of(j % mc)], j))

            def reload_v(i, cc_out, v_dst):
                # SWDGE (gpsimd) so the gather-gated reload can't convoy
                # the G stream on the shared HWDGE completion lanes; one
                # DMA per part (hi/lo interleaved per core block)
                nc.gpsimd.dma_start(
                    v_dst[i][:].rearrange("p (c m) -> p c m", c=ncores),
                    cc_out[:].rearrange("(c p) m -> p c m", p=P),
                )

            for k in range(1, ORDER):
                v_cur = v_parts
                if k < ORDER - 1:
                    v_next = [
                        vp.tile([P, w_], bf16, tag=f"v{i}", name=f"vn{i}")
                        for i, w_ in enumerate(vcols)
                    ]
                y_t = sb.tile([F, rpc], f32, tag="yT")
                js = jorder
                # hop: y_k^T = (G @ y_{k-1})^T via 3 bf16 hi/lo passes,
                # one sweep per fc chunk so partial all-gathers overlap
                # the remaining sweeps
                for i in sweep_order:
                    s, l = fchunks[i]
                    # when both hi and lo rhs fit one PSUM bank, fuse the
                    # two v_hi passes into a single 2l-column matmul and
                    # fold the halves with the epilogue DVE op instead
                    merged = 2 * l <= 512
                    hp = ps_hop.tile(
                        [F, 2 * l] if merged else [F, l],
                        f32, tag=f"hop{i}", name=f"hp{i}",
                    )
                    pinned = i == pin_i
                    for jn, j in enumerate(js):
                        if pinned:
                            g = pin[:, j * 2 * l : (j + 1) * 2 * l]
                            if k == 1:
                                nc.sync.dma_start(
                                    g, ghls[i][j * P : (j + 1) * P, :]
                                )
                        else:
                            gt = gtp.tile(
                                [P, 2 * l], bf16, tag=f"gt{i}", name="gt"
                            )
                            nc.sync.dma_start(
                                gt[:], ghls[i][j * P : (j + 1) * P, :]
                            )
                            g = gt[:]
                        gh = g[:, 0:l]
                        gl = g[:, l : 2 * l]
                        if merged:
                            nc.tensor.matmul(
                                hp[:], lhsT=v_hi(v_cur, j), rhs=g[:, 0 : 2 * l],
                                start=(jn == 0), stop=False,
                            )
                            nc.tensor.matmul(
                                hp[:, 0:l], lhsT=v_lo(v_cur, j), rhs=gh,
                                start=False, stop=(jn == jc - 1),
                            )
                        else:
                            for t, (lhs, rhs) in enumerate(
                                (
                                    (v_hi(v_cur, j), gh),
                                    (v_lo(v_cur, j), gh),
                                    (v_hi(v_cur, j), gl),
                                )
                            ):
                                nc.tensor.matmul(
                                    hp[:], lhsT=lhs, rhs=rhs,
                                    start=(jn == 0 and t == 0),
                                    stop=(jn == jc - 1 and t == 2),
                                )
                    # sweep epilogue: copy out (folding the merged
                    # halves), Wp contribution
                    if merged:
                        # walrus allows only one PSUM operand per DVE op
                        nc.vector.tensor_copy(y_t[:, s : s + l], hp[:, 0:l])
                        nc.vector.tensor_add(
                            y_t[:, s : s + l], y_t[:, s : s + l], hp[:, l : 2 * l]
                        )
                    else:
                        nc.vector.tensor_copy(y_t[:, s : s + l], hp[:])
                    pw = ps_w.tile([F, l], f32, tag="pw")
                    nc.tensor.matmul(
                        pw[:], lhsT=w_sb[:, k * F : (k + 1) * F],
                        rhs=y_t[:, s : s + l], start=True, stop=True,
                    )
                    nc.vector.tensor_add(
                        out_sb[:, s : s + l], out_sb[:, s : s + l], pw[:]
                    )
                    if k < ORDER - 1:
                        # transpose this sweep's rows to natural layout,
                        # split bf16 hi/lo, partial all-gather; the
                        # reload into the next hop's v happens there
                        m0, nm = parts[i]
                        stage = sb.tile(
                            [P, 2 * nm * F], bf16, tag=f"stage{i}",
                            name=f"stage{i}",
                        )
                        for mm in range(nm):
                            m = m0 + mm
                            tp = ps_tp.tile([P, F], f32, tag="tp", name="tp")
                            nc.tensor.transpose(
                                tp[:], y_t[:, m * P : (m + 1) * P],
                                ident[0:F, 0:F],
                            )
                            hi = stage[:, mm * F : (mm + 1) * F]
                            lo = stage[:, (nm + mm) * F : (nm + mm + 1) * F]
                            nc.vector.tensor_copy(hi, tp[:])
                            nc.vector.tensor_sub(lo, tp[:], hi)
                        cc_in = dram.tile(
                            [P, 2 * nm * F], bf16, tag=f"ccin{i}",
                            name=f"ccin{i}",
                        )
                        cc_out = dram.tile(
                            [ncores * P, 2 * nm * F], bf16, tag=f"ccout{i}",
                            name=f"ccout{i}",
                        )
                        nc.scalar.dma_start(cc_in[:], stage[:])
                        nc.gpsimd.collective_compute(
                            "AllGather",
                            mybir.AluOpType.bypass,
                            replica_groups=[list(range(ncores))],
                            ins=[cc_in.opt()],
                            outs=[cc_out.opt()],
                        )
                        reload_v(i, cc_out, v_next)
                if k < ORDER - 1:
                    v_parts = v_next

            nc.scalar.dma_start(out_t, out_sb[:])

    nc.compile()
    return nc


def get_nc(np_total=NP, ncores=NCORES):
    key = (np_total, ncores)
    if key not in _CACHE:
        _CACHE[key] = _build(np_total, ncores)
    return _CACHE[key]


def _bf16_pair(a):
    import ml_dtypes

    hi = a.astype(ml_dtypes.bfloat16)
    lo = (a - hi.astype(np.float32)).astype(ml_dtypes.bfloat16)
    return hi, lo


def prep_inputs(x, gso, weight, np_total=NP, ncores=NCORES):
    """Host-side shard prep. Returns in_maps for run_bass_kernel_spmd."""
    n = x.shape[0]
    rpc = np_total // ncores
    jc = np_total // P

    x = np.asarray(x, dtype=np.float32)
    gso = np.asarray(gso, dtype=np.float32)
    weight = np.asarray(weight, dtype=np.float32)

    wp = np.concatenate(
        [
            weight[0] - weight[2],
            weight[1] - 3.0 * weight[3],
            2.0 * weight[2],
            4.0 * weight[3],
        ],
        axis=1,
    ).astype(np.float32)  # [F, ORDER*F]

    xpad = np.zeros((np_total, F), dtype=np.float32)
    xpad[:n] = x
    gpad = np.zeros((np_total, np_total), dtype=np.float32)
    gpad[:n, :n] = gso
    g_hi, g_lo = _bf16_pair(gpad)

    # x as bf16 hi/lo pair in the per-part v layout:
    # for part (m0, nm): block col (c*nm + ml)*F + f = row (c*mc+m0+ml)*P + p
    x_hi, x_lo = _bf16_pair(xpad)
    mc = rpc // P
    parts = [(s // P, min(512, rpc - s) // P) for s in range(0, rpc, 512)]

    def part_x(m0, nm):
        # [P, (c, hi|lo, ml, f)] interleaved per core block
        hi = x_hi.reshape(ncores, mc, P, F)[:, m0 : m0 + nm].transpose(2, 0, 1, 3)
        lo = x_lo.reshape(ncores, mc, P, F)[:, m0 : m0 + nm].transpose(2, 0, 1, 3)
        return np.stack([hi, lo], axis=2).reshape(P, ncores * 2 * nm * F)

    xthl = np.ascontiguousarray(
        np.concatenate([part_x(m0, nm) for (m0, nm) in parts], axis=1)
    )

    fchunks = [(s, min(512, rpc - s)) for s in range(0, rpc, 512)]
    in_maps = []
    for c in range(ncores):
        rows = slice(c * rpc, (c + 1) * rpc)
        ght_c = g_hi[rows, :].T  # [np_total, rpc] bf16
        glt_c = g_lo[rows, :].T
        m = {"xthl": xthl, "wp": wp}
        m["xt"] = np.ascontiguousarray(xpad[rows, :].T)  # [F, rpc] fp32
        for i, (s, l) in enumerate(fchunks):
            # per-row [hi cols | lo cols] for this fc sweep
            m[f"ghl{i}"] = np.ascontiguousarray(
                np.concatenate(
                    [ght_c[:, s : s + l], glt_c[:, s : s + l]], axis=1
                )
            )
        in_maps.append(m)
    return in_maps


def assemble_output(results, n=N, ncores=NCORES):
    out_t = np.concatenate([results[c]["outT"] for c in range(ncores)], axis=1)
    return np.ascontiguousarray(out_t.T[:n]).astype(np.float32)


def kernel(x, gso, weight):
    import time

    from concourse import bass_utils

    nc = get_nc()
    in_maps = prep_inputs(x, gso, weight)
    last_err = None
    for attempt in range(3):
        try:
            res = bass_utils.run_bass_kernel_spmd(
                nc, in_maps, core_ids=list(range(NCORES))
            )
            return assemble_output(res.results)
        except Exception as e:  # transient device wedge: retry
            last_err = e
            time.sleep(5.0 * (attempt + 1))
    raise last_err



# revision 2
# speedup vs baseline: 1.5995x; 1.5995x over previous
"""ChebConv (order-4) GNN layer on 8 Trainium2 NeuronCores.

Reference computation (fp32):
    T0 = x, T1 = G x, Tk = 2 G T{k-1} - T{k-2}
    out = sum_k Tk @ W[k]          # [N, F] with N=10000, F=32

Strategy:
  * Rewrite in the power basis: y0 = x, yk = G y{k-1},
      out = sum_k yk @ Wp[k]  with
      Wp = [W0 - W2, W1 - 3 W3, 2 W2, 4 W3]   (exact modulo fp reassociation)
    so each hop is a bare matmul against G (no 2*/- epilogue).
  * Row-shard G over 8 cores (1280 padded rows each). The per-core lhsT
    tiles must hold G^T, so the host passes each core a contiguous
    transposed slice (pad N 10000 -> 10240).
  * Plain bf16 for G and v: one full-rate PE pass per hop (the rel-err
    gate is 2e-2; measured plain-bf16 error is ~3.6e-3, so the hi/lo
    compensated scheme of the earlier revision is unnecessary). fp32
    PSUM accumulation throughout.
  * Each hop runs as 3 sweeps over <=512-column chunks of yk^T. Sweeps
    0 and 1 (512 cols each, 2x 80 KiB/partition) stay PINNED in SBUF
    across all hops: their G is DMAed exactly once (during hop 1) and
    hops 2-3 replay them from SBUF. Only sweep 2 (256 cols) streams
    from DRAM each hop, and its DMA overlaps the pinned sweeps' PE
    work. Per-hop DRAM traffic after hop 1 drops to ~5 MB/core.
  * Per sweep and 128-row j-chunk: matmul (lhsT=v[j] [128,32] bf16,
    rhs=G^T tile [128,<=512] bf16) accumulates the sweep's [32,<=512]
    chunk of yk^T over all 80 j-chunks in one PSUM accumulation group.
  * The Wp contraction happens on-chip from yk^T in full fp32:
    matmul(lhsT=Wp_k [32,32], rhs=ykT chunk), DVE-add into the
    transposed output accumulator; the k=0 term uses the host xT slice.
  * After each sweep (except in the last hop), its rows are
    PE-transposed ([32,128] -> [128,32] blocks) into natural m-chunk
    layout, cast to bf16, and all-gathered in a partial collective
    (DRAM bounce) that overlaps the remaining sweeps. The reload into
    the next hop's per-part v tiles rides SWDGE (gpsimd) so the
    gather-gated DMA cannot convoy the G stream on the shared HWDGE
    completion lanes; j-chunks are consumed in gather-firing order so
    each hop starts on columns whose gather finished first.
  * Output is returned transposed ([32, 1280] per core); the host
    concatenates, transposes and drops padding.
"""

import sys

if "/opt/trn_rl_repo" not in sys.path:
    sys.path.insert(0, "/opt/trn_rl_repo")

import numpy as np

N = 10000
F = 32
ORDER = 4
NCORES = 8
P = 128
NP = 10240  # padded node count: divisible by NCORES * P
RPC = NP // NCORES  # rows per core (1280)
JC = NP // P  # global 128-row chunks (80)
MC = RPC // P  # local 128-row chunks per core (10)

SWEEPS = [512, 512, 256]  # per-hop column sweeps (each <= 512 = PSUM bank)
PINNED = (0, 1)  # sweeps whose G block stays resident in SBUF

_CACHE = {}


def _fchunks(rpc):
    out, s = [], 0
    for l in SWEEPS:
        out.append((s, l))
        s += l
    assert s == rpc
    return out


def _build(np_total, ncores):
    from concourse import bacc, masks, mybir, tile

    rpc = np_total // ncores
    jc = np_total // P
    mc = rpc // P
    f32 = mybir.dt.float32
    bf16 = mybir.dt.bfloat16
    fchunks = _fchunks(rpc)
    nfc = len(fchunks)

    nc = bacc.Bacc(
        "TRN2", target_bir_lowering=False, debug=False, num_devices=ncores
    )
    # one G^T block per fc sweep: [np_total, l] bf16
    ghls = [
        nc.dram_tensor(f"ghl{i}", [np_total, l], bf16, kind="ExternalInput").ap()
        for i, (s, l) in enumerate(fchunks)
    ]
    # per-part m-chunk geometry: part i covers m-chunks [m0, m0+nm)
    parts = [(s // P, l // P) for s, l in fchunks]
    # x in per-part v layout: block col (c*nm + ml)*F + f
    #   = padded x row (c*mc + m0 + ml)*P + p
    vcols = [ncores * nm * F for (m0, nm) in parts]
    xtv = nc.dram_tensor("xtv", [P, sum(vcols)], bf16, kind="ExternalInput").ap()
    xt = nc.dram_tensor("xt", [F, rpc], f32, kind="ExternalInput").ap()
    wp = nc.dram_tensor("wp", [F, ORDER * F], f32, kind="ExternalInput").ap()
    out_t = nc.dram_tensor("outT", [F, rpc], f32, kind="ExternalOutput").ap()

    def part_of(m):
        for i, (m0, nm) in enumerate(parts):
            if m0 <= m < m0 + nm:
                return i
        raise AssertionError

    with tile.TileContext(nc) as tc:
        with (
            tc.tile_pool(name="const", bufs=1) as constp,
            tc.tile_pool(name="gtp", bufs=16) as gtp,
            tc.tile_pool(name="vp", bufs=2) as vp,
            tc.tile_pool(name="sb", bufs=2) as sb,
            tc.tile_pool(name="ps_hop", bufs=1, space="PSUM") as ps_hop,
            tc.tile_pool(name="ps_tp", bufs=2, space="PSUM") as ps_tp,
            tc.tile_pool(name="ps_w", bufs=2, space="PSUM") as ps_w,
            tc.tile_pool(name="dram", bufs=2, space="DRAM") as dram,
        ):
            ident = constp.tile([P, P], f32)
            masks.make_identity(nc, ident[:])
            w_sb = constp.tile([F, ORDER * F], f32)
            nc.scalar.dma_start(w_sb[:], wp)
            xt_sb = constp.tile([F, rpc], f32)
            nc.scalar.dma_start(xt_sb[:], xt)
            out_sb = constp.tile([F, rpc], f32)
            pins = {
                i: constp.tile([P, jc * fchunks[i][1]], bf16, name=f"pin{i}")
                for i in PINNED
            }

            # v holds y_{k-1} as bf16, one tile per fc part so next-hop
            # matmuls only depend on the partial gather that produced
            # their columns
            v_parts = []
            off = 0
            for i, w_ in enumerate(vcols):
                vt = vp.tile([P, w_], bf16, tag=f"v{i}", name=f"v{i}")
                nc.scalar.dma_start(vt[:], xtv[:, off : off + w_])
                off += w_
                v_parts.append(vt)

            def v_of(vps, j):
                c, m = j // mc, j % mc
                i = part_of(m)
                m0, nm = parts[i]
                col = (c * nm + (m - m0)) * F
                return vps[i][:, col : col + F]

            # k = 0 contribution: out^T = Wp_0^T @ x^T (pure fp32)
            for s, l in fchunks:
                pw = ps_w.tile([F, l], f32, tag="pw")
                nc.tensor.matmul(
                    pw[:], lhsT=w_sb[:, 0:F], rhs=xt_sb[:, s : s + l],
                    start=True, stop=True,
                )
                nc.vector.tensor_copy(out_sb[:, s : s + l], pw[:])

            # j-chunks are consumed in sweep (= gather-firing) order so
            # each hop starts on columns whose gather finished first;
            # the streamed sweep stays last: its gather is smallest and
            # its consumers come after the pinned sweeps' runway
            sweep_order = list(range(nfc))
            part_rank = {i: r for r, i in enumerate(sweep_order)}
            jorder = sorted(range(jc), key=lambda j: (part_rank[part_of(j % mc)], j))

            def reload_v(i, cc_out, v_dst):
                # SWDGE (gpsimd) so the gather-gated reload can't convoy
                # the G stream on the shared HWDGE completion lanes
                nc.gpsimd.dma_start(
                    v_dst[i][:].rearrange("p (c m) -> p c m", c=ncores),
                    cc_out[:].rearrange("(c p) m -> p c m", p=P),
                )

            for k in range(1, ORDER):
                v_cur = v_parts
                if k < ORDER - 1:
                    v_next = [
                        vp.tile([P, w_], bf16, tag=f"v{i}", name=f"vn{i}")
                        for i, w_ in enumerate(vcols)
                    ]
                y_t = sb.tile([F, rpc], f32, tag="yT")
                js = jorder
                # hop: y_k^T = (G @ y_{k-1})^T, one bf16 pass; pinned
                # sweeps replay from SBUF, the streamed sweep's DMA
                # overlaps their PE work
                for i in sweep_order:
                    s, l = fchunks[i]
                    hp = ps_hop.tile([F, l], f32, tag=f"hop{i}", name=f"hp{i}")
                    pinned = i in pins
                    for jn, j in enumerate(js):
                        if pinned:
                            g = pins[i][:, j * l : (j + 1) * l]
                            if k == 1:
                                nc.sync.dma_start(
                                    g, ghls[i][j * P : (j + 1) * P, :]
                                )
                        else:
                            gt = gtp.tile([P, l], bf16, tag=f"gt{i}", name="gt")
                            nc.sync.dma_start(
                                gt[:], ghls[i][j * P : (j + 1) * P, :]
                            )
                            g = gt[:]
                        nc.tensor.matmul(
                            hp[:], lhsT=v_of(v_cur, j), rhs=g,
                            start=(jn == 0), stop=(jn == jc - 1),
                        )
                    # sweep epilogue: copy out, Wp contribution
                    nc.vector.tensor_copy(y_t[:, s : s + l], hp[:])
                    pw = ps_w.tile([F, l], f32, tag="pw")
                    nc.tensor.matmul(
                        pw[:], lhsT=w_sb[:, k * F : (k + 1) * F],
                        rhs=y_t[:, s : s + l], start=True, stop=True,
                    )
                    nc.vector.tensor_add(
                        out_sb[:, s : s + l], out_sb[:, s : s + l], pw[:]
                    )
                    if k < ORDER - 1:
                        # transpose this sweep's rows to natural layout,
                        # cast bf16, partial all-gather; the reload into
                        # the next hop's v happens in reload_v
                        m0, nm = parts[i]
                        stage = sb.tile(
                            [P, nm * F], bf16, tag=f"stage{i}",
                            name=f"stage{i}",
                        )
                        for mm in range(nm):
                            m = m0 + mm
                            tp = ps_tp.tile([P, F], f32, tag="tp", name="tp")
                            nc.tensor.transpose(
                                tp[:], y_t[:, m * P : (m + 1) * P],
                                ident[0:F, 0:F],
                            )
                            nc.vector.tensor_copy(
                                stage[:, mm * F : (mm + 1) * F], tp[:]
                            )
                        cc_in = dram.tile(
                            [P, nm * F], bf16, tag=f"ccin{i}",
                            name=f"ccin{i}",
                        )
                        cc_out = dram.tile(
                            [ncores * P, nm * F], bf16, tag=f"ccout{i}",
                            name=f"ccout{i}",
                        )
                        nc.scalar.dma_start(cc_in[:], stage[:])
                        nc.gpsimd.collective_compute(
                            "AllGather",
                            mybir.AluOpType.bypass,
                            replica_groups=[list(range(ncores))],
                            ins=[cc_in.opt()],
                            outs=[cc_out.opt()],
                        )
                        reload_v(i, cc_out, v_next)
                if k < ORDER - 1:
                    v_parts = v_next

            nc.scalar.dma_start(out_t, out_sb[:])

    nc.compile()
    return nc


def get_nc(np_total=NP, ncores=NCORES):
    key = (np_total, ncores)
    if key not in _CACHE:
        _CACHE[key] = _build(np_total, ncores)
    return _CACHE[key]


def prep_inputs(x, gso, weight, np_total=NP, ncores=NCORES):
    """Host-side shard prep. Returns in_maps for run_bass_kernel_spmd."""
    import ml_dtypes

    n = x.shape[0]
    rpc = np_total // ncores

    x = np.asarray(x, dtype=np.float32)
    gso = np.asarray(gso, dtype=np.float32)
    weight = np.asarray(weight, dtype=np.float32)

    wp = np.concatenate(
        [
            weight[0] - weight[2],
            weight[1] - 3.0 * weight[3],
            2.0 * weight[2],
            4.0 * weight[3],
        ],
        axis=1,
    ).astype(np.float32)  # [F, ORDER*F]

    xpad = np.zeros((np_total, F), dtype=np.float32)
    xpad[:n] = x
    gpad = np.zeros((np_total, np_total), dtype=np.float32)
    gpad[:n, :n] = gso
    g_bf = gpad.astype(ml_dtypes.bfloat16)

    # x as bf16 in the per-part v layout:
    # for part (m0, nm): block col (c*nm + ml)*F + f = row (c*mc+m0+ml)*P + p
    x_bf = xpad.astype(ml_dtypes.bfloat16)
    mc = rpc // P
    fchunks = _fchunks(rpc)
    parts = [(s // P, l // P) for s, l in fchunks]

    def part_x(m0, nm):
        return np.ascontiguousarray(
            x_bf.reshape(ncores, mc, P, F)[:, m0 : m0 + nm]
            .transpose(2, 0, 1, 3)
            .reshape(P, ncores * nm * F)
        )

    xtv = np.ascontiguousarray(
        np.concatenate([part_x(m0, nm) for (m0, nm) in parts], axis=1)
    )

    in_maps = []
    for c in range(ncores):
        rows = slice(c * rpc, (c + 1) * rpc)
        ght_c = g_bf[rows, :].T  # [np_total, rpc] bf16
        m = {"xtv": xtv, "wp": wp}
        m["xt"] = np.ascontiguousarray(xpad[rows, :].T)  # [F, rpc] fp32
        for i, (s, l) in enumerate(fchunks):
            m[f"ghl{i}"] = np.ascontiguousarray(ght_c[:, s : s + l])
        in_maps.append(m)
    return in_maps


def assemble_output(results, n=N, ncores=NCORES):
    out_t = np.concatenate([results[c]["outT"] for c in range(ncores)], axis=1)
    return np.ascontiguousarray(out_t.T[:n]).astype(np.float32)


def kernel(x, gso, weight):
    import time

    from concourse import bass_utils

    nc = get_nc()
    in_maps = prep_inputs(x, gso, weight)
    last_err = None
    for attempt in range(3):
        try:
            res = bass_utils.run_bass_kernel_spmd(
                nc, in_maps, core_ids=list(range(NCORES))
            )
            return assemble_output(res.results)
        except Exception as e:  # transient device wedge: retry
            last_err = e
            time.sleep(5.0 * (attempt + 1))
    raise last_err


# revision 4
# speedup vs baseline: 2.0213x; 1.2637x over previous
"""ChebConv (order-4) GNN layer on 8 Trainium2 NeuronCores.

Reference computation (fp32):
    T0 = x, T1 = G x, Tk = 2 G T{k-1} - T{k-2}
    out = sum_k Tk @ W[k]          # [N, F] with N=10000, F=32

Strategy:
  * Rewrite in the power basis: y0 = x, yk = G y{k-1},
      out = sum_k yk @ Wp[k]  with
      Wp = [W0 - W2, W1 - 3 W3, 2 W2, 4 W3]   (exact modulo fp reassociation)
    so each hop is a bare matmul against G (no 2*/- epilogue).
  * Row-shard G over 8 cores (1280 padded rows each). The per-core lhsT
    tiles must hold G^T, so the host passes each core a transposed
    slice (pad N 10000 -> 10240).
  * Plain bf16 for G and v: one full-rate PE pass per hop (the rel-err
    gate is 2e-2; measured plain-bf16 error is ~4e-3). fp32 PSUM
    accumulation throughout.
  * Each hop runs as 3 sweeps over <=512-column chunks of yk^T. Sweeps
    0 and 1 (512 cols each, 2x 80 KiB/partition) stay PINNED in SBUF
    across all hops: their G is DMAed exactly once (during hop 1) and
    hops 2-3 replay them from SBUF. Only sweep 2 (256 cols) streams
    from DRAM each hop on a separate (vector) DMA queue, overlapping
    the pinned sweeps' PE work.
  * G arrives partition-major (host pre-layout [128, jc*l]) in groups
    of 8 (pins) / 4 (stream) 128-row j-chunks per DMA, i.e. 8 KiB / 2
    KiB contiguous per partition line -- the earlier per-chunk row-major
    DMAs produced 1 KiB packets whose ~89 ns fixed cost capped HBM at
    ~220 GB/s. The streamed sweep is laid out in jorder (consumption
    order) so grouped DMAs stay contiguous.
  * Per sweep and j-chunk: matmul (lhsT=v[j] [128,32] bf16, rhs=G^T
    tile [128,<=512] bf16) accumulates the sweep's [32,<=512] chunk of
    yk^T over all 80 j-chunks in one PSUM accumulation group.
  * Output accumulates in PSUM across hops: matmul(lhsT=Wp_k bf16,
    rhs=ykT chunk bf16, start=(k==0), stop=(k==3)) into 3 dedicated
    banks; a single DVE copy + DMA at the end. y^T is cast to bf16 once
    per sweep (DVE PSUM->SBUF copy) and reused for the Wp matmul, the
    PE transposes (1 cycle/row in bf16 vs 4 for fp32) and the gather.
  * Sweep epilogues are DEFERRED: emitted after the first 8 j-matmuls
    of the following sweep so the PSUM->SBUF DVE copy latency hides
    under matmul streaming instead of stalling the PE.
  * A 2-element AllGather fires at kernel start to absorb the one-time
    collective channel setup (~43 us barrier) under hop 1's DMA phase.
  * After each sweep (except in the last hop), its rows are
    PE-transposed ([32,128] -> [128,32] blocks) into natural m-chunk
    layout and all-gathered in a partial collective (DRAM bounce) that
    overlaps the remaining sweeps. The reload into the next hop's
    per-part v tiles rides SWDGE (gpsimd) so the gather-gated DMA
    cannot convoy the G stream on the HWDGE lanes; j-chunks are
    consumed in gather-firing order so each hop starts on columns
    whose gather finished first.
  * Output is returned transposed ([32, 1280] per core); the host
    concatenates, transposes and drops padding.
"""

import sys

if "/opt/trn_rl_repo" not in sys.path:
    sys.path.insert(0, "/opt/trn_rl_repo")

import numpy as np

N = 10000
F = 32
ORDER = 4
NCORES = 8
P = 128
NP = 10240  # padded node count: divisible by NCORES * P
RPC = NP // NCORES  # rows per core (1280)
JC = NP // P  # global 128-row chunks (80)
MC = RPC // P  # local 128-row chunks per core (10)

SWEEPS = [512, 512, 256]  # per-hop column sweeps (each <= 512 = PSUM bank)
PINNED = (0, 1)  # sweeps whose G block stays resident in SBUF
PIN_GROUP = 8  # j-chunks per pin-load DMA (8 KiB per partition line)
STREAM_GROUP = 4  # j-chunks per stream DMA (2 KiB per partition line)
STREAM_BUFS = 8  # stream tile pool depth (prefetch 32 j-chunks)
DEFER = 8  # j-matmuls of the next sweep before the deferred epilogue

_CACHE = {}


def _fchunks(rpc):
    out, s = [], 0
    for l in SWEEPS:
        out.append((s, l))
        s += l
    assert s == rpc
    return out


def _geometry(rpc):
    fchunks = _fchunks(rpc)
    parts = [(s // P, l // P) for s, l in fchunks]
    mc = rpc // P

    def part_of(m):
        for i, (m0, nm) in enumerate(parts):
            if m0 <= m < m0 + nm:
                return i
        raise AssertionError

    jc = NP // P
    jorder = sorted(range(jc), key=lambda j: (part_of(j % mc), j))
    return fchunks, parts, part_of, jorder


def _build(np_total, ncores):
    from concourse import bacc, masks, mybir, tile

    rpc = np_total // ncores
    jc = np_total // P
    mc = rpc // P
    f32 = mybir.dt.float32
    bf16 = mybir.dt.bfloat16
    fchunks, parts, part_of, jorder = _geometry(rpc)
    nfc = len(fchunks)
    stream_i = [i for i in range(nfc) if i not in PINNED]
    assert len(stream_i) == 1
    stream_i = stream_i[0]

    nc = bacc.Bacc(
        "TRN2", target_bir_lowering=False, debug=False, num_devices=ncores
    )
    # one G^T block per fc sweep, partition-major: [128, jc*l]; the
    # streamed sweep is laid out in jorder
    ghls = [
        nc.dram_tensor(f"ghl{i}", [P, jc * l], bf16, kind="ExternalInput").ap()
        for i, (s, l) in enumerate(fchunks)
    ]
    # x in per-part v layout: block col (c*nm + ml)*F + f
    #   = padded x row (c*mc + m0 + ml)*P + p
    vcols = [ncores * nm * F for (m0, nm) in parts]
    xtv = nc.dram_tensor("xtv", [P, sum(vcols)], bf16, kind="ExternalInput").ap()
    xtb = nc.dram_tensor("xtb", [F, rpc], bf16, kind="ExternalInput").ap()
    wpb = nc.dram_tensor("wpb", [F, ORDER * F], bf16, kind="ExternalInput").ap()
    out_t = nc.dram_tensor("outT", [F, rpc], f32, kind="ExternalOutput").ap()

    with tile.TileContext(nc) as tc:
        with (
            tc.tile_pool(name="const", bufs=1) as constp,
            tc.tile_pool(name="gtp", bufs=STREAM_BUFS) as gtp,
            tc.tile_pool(name="vp", bufs=2) as vp,
            tc.tile_pool(name="sb", bufs=2) as sb,
            tc.tile_pool(name="ps_hop", bufs=1, space="PSUM") as ps_hop,
            tc.tile_pool(name="ps_tp", bufs=2, space="PSUM") as ps_tp,
            tc.tile_pool(name="ps_out", bufs=1, space="PSUM") as ps_out,
            tc.tile_pool(name="dram", bufs=2, space="DRAM") as dram,
        ):
            identb = constp.tile([P, P], bf16)
            masks.make_identity(nc, identb[:])
            wpb_sb = constp.tile([F, ORDER * F], bf16)
            nc.scalar.dma_start(wpb_sb[:], wpb)
            xtb_sb = constp.tile([F, rpc], bf16)
            nc.scalar.dma_start(xtb_sb[:], xtb)
            out_sb = constp.tile([F, rpc], f32)
            pins = {
                i: constp.tile([P, jc * fchunks[i][1]], bf16, name=f"pin{i}")
                for i in PINNED
            }

            # prime the collective channel: absorb the one-time barrier
            # under hop 1's DMA phase
            pr_in = dram.tile([1, 2], bf16, tag="pr_in", name="pr_in")
            pr_out = dram.tile([ncores, 2], bf16, tag="pr_out", name="pr_out")
            nc.scalar.dma_start(pr_in[:], xtv[0:1, 0:2])
            nc.gpsimd.collective_compute(
                "AllGather",
                mybir.AluOpType.bypass,
                replica_groups=[list(range(ncores))],
                ins=[pr_in.opt()],
                outs=[pr_out.opt()],
            )

            # v holds y_{k-1} as bf16, one tile per fc part so next-hop
            # matmuls only depend on the partial gather that produced
            # their columns
            v_parts = []
            off = 0
            for i, w_ in enumerate(vcols):
                vt = vp.tile([P, w_], bf16, tag=f"v{i}", name=f"v{i}")
                nc.scalar.dma_start(vt[:], xtv[:, off : off + w_])
                off += w_
                v_parts.append(vt)

            def v_of(vps, j):
                c, m = j // mc, j % mc
                i = part_of(m)
                m0, nm = parts[i]
                col = (c * nm + (m - m0)) * F
                return vps[i][:, col : col + F]

            # pinned-sweep loads: fat grouped DMAs, natural j order
            for i in PINNED:
                l = fchunks[i][1]
                for g0 in range(0, jc, PIN_GROUP):
                    w_ = min(PIN_GROUP, jc - g0) * l
                    nc.sync.dma_start(
                        pins[i][:, g0 * l : g0 * l + w_],
                        ghls[i][:, g0 * l : g0 * l + w_],
                    )

            # output accumulates in PSUM across all hops (one group per
            # sweep chunk); k = 0 term opens the group from bf16 x^T
            out_banks = []
            for i, (s, l) in enumerate(fchunks):
                ob = ps_out.tile([F, l], f32, tag=f"out{i}", name=f"out{i}")
                nc.tensor.matmul(
                    ob[:], lhsT=wpb_sb[:, 0:F], rhs=xtb_sb[:, s : s + l],
                    start=True, stop=False,
                )
                out_banks.append(ob)

            def reload_v(i, cc_out, v_dst):
                # SWDGE (gpsimd) so the gather-gated reload can't convoy
                # the G stream on the HWDGE completion lanes
                nc.gpsimd.dma_start(
                    v_dst[i][:].rearrange("p (c m) -> p c m", c=ncores),
                    cc_out[:].rearrange("(c p) m -> p c m", p=P),
                )

            pending = []

            def flush_pending():
                for f in pending:
                    f()
                pending.clear()

            def make_epilogue(k, i, hp, y_bf, v_next):
                s, l = fchunks[i]

                def epi():
                    nc.vector.tensor_copy(y_bf[:, s : s + l], hp[:])
                    nc.tensor.matmul(
                        out_banks[i][:],
                        lhsT=wpb_sb[:, k * F : (k + 1) * F],
                        rhs=y_bf[:, s : s + l],
                        start=False, stop=(k == ORDER - 1),
                    )
                    if k < ORDER - 1:
                        m0, nm = parts[i]
                        stage = sb.tile(
                            [P, nm * F], bf16, tag=f"stage{i}",
                            name=f"stage{i}",
                        )
                        for mm in range(nm):
                            m = m0 + mm
                            tp = ps_tp.tile([P, F], bf16, tag="tp", name="tp")
                            nc.tensor.transpose(
                                tp[:], y_bf[:, m * P : (m + 1) * P],
                                identb[0:F, 0:F],
                            )
                            nc.vector.tensor_copy(
                                stage[:, mm * F : (mm + 1) * F], tp[:]
                            )
                        cc_in = dram.tile(
                            [P, nm * F], bf16, tag=f"ccin{i}", name=f"ccin{i}"
                        )
                        cc_out = dram.tile(
                            [ncores * P, nm * F], bf16, tag=f"ccout{i}",
                            name=f"ccout{i}",
                        )
                        nc.scalar.dma_start(cc_in[:], stage[:])
                        nc.gpsimd.collective_compute(
                            "AllGather",
                            mybir.AluOpType.bypass,
                            replica_groups=[list(range(ncores))],
                            ins=[cc_in.opt()],
                            outs=[cc_out.opt()],
                        )
                        reload_v(i, cc_out, v_next)

                return epi

            for k in range(1, ORDER):
                v_cur = v_parts
                v_next = None
                if k < ORDER - 1:
                    v_next = [
                        vp.tile([P, w_], bf16, tag=f"v{i}", name=f"vn{i}")
                        for i, w_ in enumerate(vcols)
                    ]
                y_bf = sb.tile([F, rpc], bf16, tag="yT")
                for i in range(nfc):
                    s, l = fchunks[i]
                    hp = ps_hop.tile([F, l], f32, tag=f"hop{i}", name=f"hp{i}")
                    pinned = i in pins
                    # hop 1 consumes pinned sweeps in natural j order to
                    # chase the grouped pin DMAs; everything else runs in
                    # jorder (gather-firing order)
                    js = list(range(jc)) if (pinned and k == 1) else jorder
                    for jn, j in enumerate(js):
                        if pinned:
                            g = pins[i][:, j * l : (j + 1) * l]
                        else:
                            rank = jn  # stream layout is jorder-major
                            gi, go = rank // STREAM_GROUP, rank % STREAM_GROUP
                            if go == 0:
                                w_ = min(STREAM_GROUP, jc - rank) * l
                                gt = gtp.tile(
                                    [P, STREAM_GROUP * l], bf16,
                                    tag="gt", name="gt",
                                )
                                nc.scalar.dma_start(
                                    gt[:, 0:w_],
                                    ghls[i][:, rank * l : rank * l + w_],
                                )
                            g = gt[:, go * l : (go + 1) * l]
                        nc.tensor.matmul(
                            hp[:], lhsT=v_of(v_cur, j), rhs=g,
                            start=(jn == 0), stop=(jn == jc - 1),
                        )
                        if jn == DEFER - 1:
                            flush_pending()
                    pending.append(make_epilogue(k, i, hp, y_bf, v_next))
                if k < ORDER - 1:
                    v_parts = v_next
            flush_pending()

            for i, (s, l) in enumerate(fchunks):
                nc.vector.tensor_copy(out_sb[:, s : s + l], out_banks[i][:])
            nc.sync.dma_start(out_t, out_sb[:])

    nc.compile()
    return nc


def get_nc(np_total=NP, ncores=NCORES):
    key = (np_total, ncores)
    if key not in _CACHE:
        _CACHE[key] = _build(np_total, ncores)
    return _CACHE[key]


def prep_inputs(x, gso, weight, np_total=NP, ncores=NCORES):
    """Host-side shard prep. Returns in_maps for run_bass_kernel_spmd."""
    import ml_dtypes

    n = x.shape[0]
    rpc = np_total // ncores
    jc = np_total // P

    x = np.asarray(x, dtype=np.float32)
    gso = np.asarray(gso, dtype=np.float32)
    weight = np.asarray(weight, dtype=np.float32)

    wp = np.concatenate(
        [
            weight[0] - weight[2],
            weight[1] - 3.0 * weight[3],
            2.0 * weight[2],
            4.0 * weight[3],
        ],
        axis=1,
    ).astype(ml_dtypes.bfloat16)  # [F, ORDER*F]

    xpad = np.zeros((np_total, F), dtype=np.float32)
    xpad[:n] = x
    gpad = np.zeros((np_total, np_total), dtype=np.float32)
    gpad[:n, :n] = gso
    g_bf = gpad.astype(ml_dtypes.bfloat16)

    x_bf = xpad.astype(ml_dtypes.bfloat16)
    mc = rpc // P
    fchunks, parts, part_of, jorder = _geometry(rpc)

    def part_x(m0, nm):
        return np.ascontiguousarray(
            x_bf.reshape(ncores, mc, P, F)[:, m0 : m0 + nm]
            .transpose(2, 0, 1, 3)
            .reshape(P, ncores * nm * F)
        )

    xtv = np.ascontiguousarray(
        np.concatenate([part_x(m0, nm) for (m0, nm) in parts], axis=1)
    )

    in_maps = []
    for c in range(ncores):
        rows = slice(c * rpc, (c + 1) * rpc)
        ght_c = g_bf[rows, :].T  # [np_total, rpc] bf16
        m = {"xtv": xtv, "wpb": wp}
        m["xtb"] = np.ascontiguousarray(
            xpad[rows, :].T.astype(ml_dtypes.bfloat16)
        )
        for i, (s, l) in enumerate(fchunks):
            # partition-major: [128, jc*l]; streamed sweep in jorder
            blk = ght_c[:, s : s + l].reshape(jc, P, l)
            if i not in PINNED:
                blk = blk[jorder]
            m[f"ghl{i}"] = np.ascontiguousarray(
                blk.transpose(1, 0, 2).reshape(P, jc * l)
            )
        in_maps.append(m)
    return in_maps


def assemble_output(results, n=N, ncores=NCORES):
    out_t = np.concatenate([results[c]["outT"] for c in range(ncores)], axis=1)
    return np.ascontiguousarray(out_t.T[:n]).astype(np.float32)


def kernel(x, gso, weight):
    import time

    from concourse import bass_utils

    nc = get_nc()
    in_maps = prep_inputs(x, gso, weight)
    last_err = None
    for attempt in range(3):
        try:
            res = bass_utils.run_bass_kernel_spmd(
                nc, in_maps, core_ids=list(range(NCORES))
            )
            return assemble_output(res.results)
        except Exception as e:  # transient device wedge: retry
            last_err = e
            time.sleep(5.0 * (attempt + 1))
    raise last_err
